# revision 59
# baseline (speedup 1.0000x reference)
"""Trainium2 Bass kernel for nn_CopiedSetEncoder (set encoder with recurrent
attention). Self-contained: shards batch across 8 NeuronCores, builds a
length-specialized SPMD Tile kernel, runs it, and reassembles the output.

Design notes (v2):
- Tokens are packed contiguously per core (no per-slot 128-alignment); LPT
  assignment balances total tokens per core. Chunk masks handle slot
  membership, so a 128-token chunk may span slots.
- Softmax weights are kept in bf16 (fp32 exponent range -> exp() cannot
  overflow), which allows using the *unnormalized* exp(logits) directly as
  the moving operand of a transposed attended accumulation:
      attT[e, j]  += embB1[tok, e]^T @ w[tok, j]   (per 128-token chunk)
  with a ones-column appended to embB1 so the softmax denominator S_j falls
  out of the same accumulation. This removes the 1-col S matmul chain and
  the weight-normalization pass; normalization happens on [128, 2, 16].
- All activations (relu/exp/tanh/copy) live in one activation table;
  sigmoid is computed as 0.5*tanh(0.5x)+0.5 to avoid 1.3us table swaps.
- xT is DMA'd per 512-token tile so the first MLP tile starts early.
"""
import numpy as np
import ml_dtypes

import concourse.bass as bass
import concourse.mybir as mybir
import concourse.tile as tile
from concourse.bass_utils import run_bass_kernel_spmd

B, F_, D_IN = 128, 1024, 128
H1, H2, E, H = 512, 512, 256, 256
N_SHUFFLE = 5
NCORES = 8
BLOC = B // NCORES  # 16 batch slots per core
NEG = -1e30
C1 = 15.0  # fixed logit shift

f32 = mybir.dt.float32
f16 = mybir.dt.float16
bf16 = mybir.dt.bfloat16


def _split_multi_waits(nc):
    """HW allows at most one sync wait per instruction; hoist extras into
    standalone InstEventSemaphore carriers on the same engine."""
    cnt = 0
    for bb in nc.main_func.blocks:
        insts = bb.instructions  # live list
        i = 0
        while i < len(insts):
            ins = insts[i]
            si = ins.sync_info
            if si is not None and si.on_wait and len(si.on_wait) > 1:
                waits = list(si.on_wait)
                carriers = []
                for w in waits[:-1]:
                    cnt += 1
                    ev = mybir.InstEventSemaphore(name=f"wsplit-{cnt}")
                    ev.engine = ins.engine
                    ev.sync_info = mybir.SyncInfo(on_wait=[w], on_update=[])
                    carriers.append(ev)
                ins.sync_info = mybir.SyncInfo(
                    on_wait=[waits[-1]], on_update=list(si.on_update)
                )
                for j, ev in enumerate(carriers):
                    insts.insert(i + j, ev)
                    nc.register_instruction(ev, overwrite=True)
                i += len(carriers)
            i += 1
    return cnt


def _xt_groups(n_tiles):
    """Tile-group layout for the xT DMAs: first two tiles land alone so the
    MLP can start early, the rest in batches of five."""
    groups = [[0], [1]] if n_tiles >= 2 else [[0]]
    s = len(groups)
    while s < n_tiles:
        groups.append(list(range(s, min(s + 5, n_tiles))))
        s = groups[-1][-1] + 1
    return groups


def _build_module(n_tiles):
    """One SPMD program for all cores; t_common = n_tiles*512 packed tokens."""
    nc = bass.Bass()
    t_common = n_tiles * 512
    tot_chunks = n_tiles * 4
    groups = _xt_groups(n_tiles)

    # ---- inputs ----
    xt_es = [
        nc.declare_dram_parameter(
            f"xtg{gi}", [128, len(g) * 512], f16, isOutput=False
        )
        for gi, g in enumerate(groups)
    ]
    w1_e = nc.declare_dram_parameter("w1", [128, H1], f16, isOutput=False)
    w2_e = nc.declare_dram_parameter("w2", [128, 4, H2], f16, isOutput=False)
    w3_e = nc.declare_dram_parameter("w3", [128, 4, E], f16, isOutput=False)
    wih_e = nc.declare_dram_parameter("wih", [128, 2, 4 * H], f16, isOutput=False)
    whh_e = nc.declare_dram_parameter("whh", [128, 2, 4 * H], f16, isOutput=False)
    b1_e = nc.declare_dram_parameter("b1", [128, 4], f32, isOutput=False)
    b2_e = nc.declare_dram_parameter("b2", [128, 4], f32, isOutput=False)
    bg16_e = nc.declare_dram_parameter("bg16", [1, 8 * 128], f16, isOutput=False)
    mask_e = nc.declare_dram_parameter(
        "mask", [128, tot_chunks, BLOC], f32, isOutput=False
    )
    w0T_e = nc.declare_dram_parameter(
        "w0T", [128, tot_chunks, BLOC], f16, isOutput=False
    )
    ones1_e = nc.declare_dram_parameter("ones1", [1, 128], f32, isOutput=False)
    ident_e = nc.declare_dram_parameter("ident", [128, 128], f32, isOutput=False)
    ident16_e = nc.declare_dram_parameter("ident16", [128, 128], f16, isOutput=False)
    att_o = nc.declare_dram_parameter("att", [BLOC, E], f32, isOutput=True)
    qt_o = nc.declare_dram_parameter("qt", [BLOC, H], f32, isOutput=True)

    with tile.TileContext(nc) as tc:
        with tc.tile_pool(name="big", bufs=1) as big, \
             tc.tile_pool(name="wp", bufs=1) as wp:
            # resident tensors; xts[t] = (group tile, token offset)
            xtg = []
            for gi, g in enumerate(groups):
                xt_t = big.tile([128, len(g) * 512], f16, name=f"xtsg{gi}")
                xtg.append(xt_t)
            xts = {}
            for gi, g in enumerate(groups):
                for loc, t in enumerate(g):
                    xts[t] = (xtg[gi], loc * 512)
            embA = big.tile([128, 2, t_common], f16)
            embB = big.tile([128, tot_chunks, 256], f16)
            w1T = big.tile([128, tot_chunks, BLOC], f32)
            wTn = big.tile([128, tot_chunks, BLOC], f16)
            w1 = wp.tile([128, H1], f16)
            w2 = wp.tile([128, 4, H2], f16)
            w3 = wp.tile([128, 4, E], f16)
            wih = wp.tile([128, 2, 4 * H], f16)
            whh = wp.tile([128, 2, 4 * H], f16)
            b1 = wp.tile([128, 4], f32)
            b2 = wp.tile([128, 4], f32)
            bg16 = wp.tile([1, 8 * 128], f16)
            mask = wp.tile([128, tot_chunks, BLOC], f32)
            w0T = wp.tile([128, tot_chunks, BLOC], f16)
            ones1 = wp.tile([1, 128], f32)
            ident = wp.tile([128, 128], f32)
            ident16 = wp.tile([128, 128], f16)
            # DMA order: what the first MLP tiles need lands first; the
            # attention-phase tensors (wih/whh/mask/w0T) land last
            nc.sync.dma_start(out=xtg[0][:], in_=xt_es[0][:])
            nc.sync.dma_start(out=w1[:], in_=w1_e[:])
            nc.sync.dma_start(out=b1[:], in_=b1_e[:])
            if len(groups) > 1:
                nc.sync.dma_start(out=xtg[1][:], in_=xt_es[1][:])
            for dst, src in [(w2, w2_e), (b2, b2_e), (w3, w3_e)]:
                nc.sync.dma_start(out=dst[:], in_=src[:])
            for gi in range(2, len(groups)):
                nc.sync.dma_start(out=xtg[gi][:], in_=xt_es[gi][:])
            for dst, src in [
                (ident16, ident16_e), (ident, ident_e), (ones1, ones1_e),
                (bg16, bg16_e), (wih, wih_e), (whh, whh_e), (mask, mask_e),
                (w0T, w0T_e),
            ]:
                nc.sync.dma_start(out=dst[:], in_=src[:])

            # warm the PE p-state while the first input DMAs are in flight
            with tc.tile_pool(name="warm", bufs=1) as wmp, \
                 tc.tile_pool(name="psW", bufs=1, space="PSUM") as psW:
                wrm = wmp.tile([128, 512], f16)
                nc.vector.memset(wrm[:], 0.0)
                pw = psW.tile([128, 512], f32)
                for k in range(8):
                    nc.tensor.matmul(
                        pw[:], wrm[:, 0:128], wrm[:],
                        start=(k == 0), stop=(k == 7),
                    )

            # ---- phase 1: MLP over 512-token tiles ----
            with tc.tile_pool(name="mlp", bufs=3) as mp, \
                 tc.tile_pool(name="ps1", bufs=2, space="PSUM") as ps1, \
                 tc.tile_pool(name="ps2", bufs=2, space="PSUM") as ps2, \
                 tc.tile_pool(name="ps3", bufs=2, space="PSUM") as ps3:
                for t in range(n_tiles):
                    sl = slice(t * 512, (t + 1) * 512)
                    xt_t, xoff = xts[t]
                    h1t = mp.tile([128, 4, 512], f16, tag="h1")
                    for mc in range(4):
                        p = ps1.tile([128, 512], f32, tag="pA")
                        nc.tensor.matmul(
                            p[:], w1[:, mc * 128:(mc + 1) * 128],
                            xt_t[:, xoff:xoff + 512],
                            start=True, stop=True,
                        )
                        if mc % 2 == 0:
                            nc.scalar.activation(
                                out=h1t[:, mc, :], in_=p[:],
                                func=mybir.ActivationFunctionType.Relu,
                                bias=b1[:, mc:mc + 1], scale=1.0,
                            )
                        else:
                            nc.vector.tensor_scalar(
                                out=h1t[:, mc, :], in0=p[:], scalar1=b1[:, mc:mc + 1],
                                scalar2=0.0, op0=mybir.AluOpType.add,
                                op1=mybir.AluOpType.max,
                            )
                    h2t = mp.tile([128, 4, 512], f16, tag="h2")
                    for mc in range(4):
                        p = ps2.tile([128, 512], f32, tag="pB")
                        for kc in range(4):
                            nc.tensor.matmul(
                                p[:], w2[:, kc, mc * 128:(mc + 1) * 128],
                                h1t[:, kc, :], start=(kc == 0), stop=(kc == 3),
                            )
                        if mc % 2 == 0:
                            nc.scalar.activation(
                                out=h2t[:, mc, :], in_=p[:],
                                func=mybir.ActivationFunctionType.Relu,
                                bias=b2[:, mc:mc + 1], scale=1.0,
                            )
                        else:
                            nc.vector.tensor_scalar(
                                out=h2t[:, mc, :], in0=p[:], scalar1=b2[:, mc:mc + 1],
                                scalar2=0.0, op0=mybir.AluOpType.add,
                                op1=mybir.AluOpType.max,
                            )
                    # embA: [e-chunk partitions, tokens]
                    for mc in range(2):
                        p = ps3.tile([128, 512], f32, tag="pC")
                        for kc in range(4):
                            nc.tensor.matmul(
                                p[:], w3[:, kc, mc * 128:(mc + 1) * 128],
                                h2t[:, kc, :], start=(kc == 0), stop=(kc == 3),
                            )
                        nc.scalar.copy(out=embA[:, mc, sl], in_=p[:])
                    # embB: [token partitions, e] via PE transposes of embA
                    for tb in range(4):
                        cch = t * 4 + tb
                        tsl = slice(t * 512 + tb * 128, t * 512 + (tb + 1) * 128)
                        for hh in range(2):
                            pt = ps3.tile([128, 128], f16, tag="pT")
                            nc.tensor.transpose(
                                pt[:], embA[:, hh, tsl], ident16[:, :]
                            )
                            dst = embB[:, cch, hh * 128:(hh + 1) * 128]
                            if (tb + hh) % 2 == 0:
                                nc.scalar.copy(out=dst, in_=pt[:])
                            else:
                                nc.vector.tensor_copy(dst, pt[:])

            # ---- phase 2: recurrent attention ----
            with tc.tile_pool(name="att", bufs=1) as ap, \
                 tc.tile_pool(name="attd", bufs=3) as ad, \
                 tc.tile_pool(name="psL", bufs=2, space="PSUM") as psL, \
                 tc.tile_pool(name="psA", bufs=1, space="PSUM") as psA, \
                 tc.tile_pool(name="psS", bufs=1, space="PSUM") as psS, \
                 tc.tile_pool(name="psG", bufs=1, space="PSUM") as psG, \
                 tc.tile_pool(name="psT", bufs=1, space="PSUM") as psT:
                qtT = ap.tile([128, 2, BLOC], f16)      # query, [h, b]
                qtT32 = ap.tile([128, 2, BLOC], f32)
                ct = ap.tile([128, 2, BLOC], f32)       # cell state
                attTn = ap.tile([128, 2, BLOC], f16)    # normalized attended^T
                attT32 = ap.tile([128, 2, BLOC], f32)
                rRow = ap.tile([1, BLOC], f32)
                rB = ap.tile([128, BLOC], f32)
                onesc = ap.tile([128, 1], f32)
                onesr = ap.tile([1, BLOC], f16)
                att_sb = ap.tile([BLOC, E], f32)
                qt_out = ap.tile([BLOC, H], f32)
                colg = ap.tile([128, (tot_chunks + 7) // 8, BLOC], f32)
                colS = ap.tile([128, BLOC], f32)
                nc.vector.memset(qtT[:], 0.0)
                nc.vector.memset(ct[:], 0.0)
                nc.vector.memset(onesc[:], 1.0)
                nc.vector.memset(onesr[:], 1.0)

                n_grp = (tot_chunks + 7) // 8
                for it in range(N_SHUFFLE):
                    if it > 0:
                        # logits token-major: chunk-stationary matmuls,
                        # grouped so mask+exp pipeline under the matmuls;
                        # per-group column sums run on the DVE via a
                        # chunk-transposed strided view
                        for g in range(n_grp):
                            nch = min(8, tot_chunks - g * 8)
                            lgp = psL.tile([128, 8, BLOC], f32, tag="lgp")
                            for ci in range(nch):
                                c = g * 8 + ci
                                for kc in range(2):
                                    nc.tensor.matmul(
                                        lgp[:, ci, :],
                                        embA[:, kc, c * 128:(c + 1) * 128],
                                        qtT[:, kc, :],
                                        start=(kc == 0), stop=(kc == 1),
                                    )
                            lgs = ad.tile([128, 8, BLOC], f32, tag="lgs")
                            nc.vector.tensor_tensor(
                                out=lgs[:, :nch, :], in0=lgp[:, :nch, :],
                                in1=mask[:, g * 8:g * 8 + nch, :],
                                op=mybir.AluOpType.add,
                            )
                            nc.scalar.activation(
                                out=w1T[:, g * 8:g * 8 + nch, :],
                                in_=lgs[:, :nch, :],
                                func=mybir.ActivationFunctionType.Exp,
                            )
                            ws = w1T[:, g * 8:g * 8 + nch, :]
                            wv = bass.AP(
                                tensor=ws.tensor, offset=ws.offset,
                                ap=[list(ws.ap[0]), list(ws.ap[2]),
                                    list(ws.ap[1])],
                            )
                            nc.vector.tensor_reduce(
                                out=colg[:, g, :], in_=wv,
                                axis=mybir.AxisListType.X,
                                op=mybir.AluOpType.add,
                            )
                            # accumulate into colS incrementally (pipelined
                            # under the next group's matmuls)
                            if g == 0:
                                nc.vector.tensor_copy(colS[:], colg[:, 0, :])
                            else:
                                nc.vector.tensor_tensor(
                                    out=colS[:], in0=colS[:], in1=colg[:, g, :],
                                    op=mybir.AluOpType.add,
                                )
                        # partition sum via one matmul -> S^T [1, BLOC]
                        sT_ps = psS.tile([1, BLOC], f32, tag="sT")
                        nc.tensor.matmul(
                            sT_ps[:, :], onesc[:], colS[:], start=True, stop=True
                        )
                        nc.vector.reciprocal(rRow[:], sT_ps[:])
                        rB_ps = psT.tile([128, BLOC], f32, tag="rbp")
                        nc.tensor.matmul(
                            rB_ps[:], ones1[:], rRow[:], start=True, stop=True
                        )
                        # normalized fp16 weights, quarter-split so the
                        # attended chain starts early; rB read from PSUM
                        rb_ap = rB_ps[:]
                        qs = [
                            (k * tot_chunks) // 4 for k in range(5)
                        ]
                        for k in range(4):
                            lo, hi = qs[k], qs[k + 1]
                            rB_bk = bass.AP(
                                tensor=rb_ap.tensor, offset=rb_ap.offset,
                                ap=[list(rb_ap.ap[0]), [0, hi - lo],
                                    list(rb_ap.ap[1])],
                            )
                            nc.vector.tensor_tensor(
                                out=wTn[:, lo:hi, :], in0=w1T[:, lo:hi, :],
                                in1=rB_bk, op=mybir.AluOpType.mult,
                            )
                        wsrc = wTn
                    else:
                        wsrc = w0T

                    # attended (transposed): attT[e,j] += embB^T @ wn, two
                    # contiguous single-region accumulation chains —
                    # interleaving PSUM regions per-instruction stalls the PE.
                    paT = psA.tile([128, 2, BLOC], f32)
                    for half in range(2):
                        for c in range(tot_chunks):
                            nc.tensor.matmul(
                                paT[:, half, :],
                                embB[:, c, half * 128:(half + 1) * 128],
                                wsrc[:, c, :],
                                start=(c == 0), stop=(c == tot_chunks - 1),
                            )
                    nc.vector.tensor_copy(attTn[:], paT[:])
                    if it == N_SHUFFLE - 1:
                        # att output: transpose + DMA now, overlapping the
                        # final LSTM update
                        nc.vector.tensor_copy(attT32[:], paT[:])
                        for c in range(2):
                            pta2 = psT.tile([BLOC, 128], f32, tag="ptq")
                            nc.tensor.transpose(
                                pta2[:], attT32[:, c, :], ident[:, :]
                            )
                            nc.vector.tensor_copy(
                                att_sb[:, c * 128:(c + 1) * 128], pta2[:]
                            )
                        nc.sync.dma_start(out=att_o[:], in_=att_sb[:])

                    # LSTM gates = bias + Whh @ qt + Wih @ att
                    g_ps = psG.tile([128, 8, BLOC], f32)
                    for mc in range(8):
                        msl = slice(mc * 128, (mc + 1) * 128)
                        nc.tensor.matmul(
                            g_ps[:, mc, :], bg16[0:1, msl], onesr[:],
                            start=True, stop=False,
                        )
                        if it > 0:  # qtT == 0 at it 0
                            for kc in range(2):
                                nc.tensor.matmul(
                                    g_ps[:, mc, :], whh[:, kc, msl],
                                    qtT[:, kc, :],
                                    start=False, stop=False,
                                )
                        for kc in range(2):
                            nc.tensor.matmul(
                                g_ps[:, mc, :], wih[:, kc, msl], attTn[:, kc, :],
                                start=False, stop=(kc == 1),
                            )
                    # gates host-permuted to [i, f, o, g] with the sigmoid
                    # gates' weights pre-halved, so one tanh covers all 8
                    # chunks; sigmoid(x) = 0.5*tanh(0.5x)+0.5
                    gact = ad.tile([128, 8, BLOC], f32, tag="gact")
                    nc.scalar.activation(
                        out=gact[:], in_=g_ps[:],
                        func=mybir.ActivationFunctionType.Tanh, scale=1.0,
                    )
                    nc.vector.tensor_scalar(
                        out=gact[:, 0:6, :], in0=gact[:, 0:6, :],
                        scalar1=0.5, scalar2=0.5,
                        op0=mybir.AluOpType.mult, op1=mybir.AluOpType.add,
                    )
                    if it == 0:
                        # ct == 0: ct = i*g directly
                        nc.vector.tensor_tensor(
                            out=ct[:], in0=gact[:, 0:2, :], in1=gact[:, 6:8, :],
                            op=mybir.AluOpType.mult,
                        )
                    else:
                        tmp = ad.tile([128, 2, BLOC], f32, tag="tmp")
                        nc.vector.tensor_tensor(
                            out=tmp[:], in0=gact[:, 0:2, :],
                            in1=gact[:, 6:8, :], op=mybir.AluOpType.mult,
                        )
                        nc.vector.tensor_tensor(
                            out=ct[:], in0=gact[:, 2:4, :], in1=ct[:],
                            op=mybir.AluOpType.mult,
                        )
                        nc.vector.tensor_tensor(
                            out=ct[:], in0=ct[:], in1=tmp[:],
                            op=mybir.AluOpType.add,
                        )
                    th = ad.tile([128, 2, BLOC], f32, tag="th")
                    nc.scalar.activation(
                        out=th[:], in_=ct[:],
                        func=mybir.ActivationFunctionType.Tanh,
                    )
                    if it == N_SHUFFLE - 1:
                        nc.vector.tensor_tensor(
                            out=qtT32[:], in0=gact[:, 4:6, :], in1=th[:],
                            op=mybir.AluOpType.mult,
                        )
                    else:
                        nc.vector.tensor_tensor(
                            out=qtT[:], in0=gact[:, 4:6, :], in1=th[:],
                            op=mybir.AluOpType.mult,
                        )

                # qt output: transpose qtT32 (f32) to [16, 256]
                for c in range(2):
                    ptq = psT.tile([BLOC, 128], f32, tag="ptq")
                    nc.tensor.transpose(ptq[:], qtT32[:, c, :], ident[:, :])
                    nc.vector.tensor_copy(qt_out[:, c * 128:(c + 1) * 128], ptq[:])
                nc.sync.dma_start(out=qt_o[:], in_=qt_out[:])

    _split_multi_waits(nc)
    return nc


def kernel(state, length, W1, b1, W2, b2, W3, b3, W_ih, W_hh, b_ih, b_hh):
    state = np.asarray(state, dtype=np.float32)
    length = np.asarray(length, dtype=np.int32)
    lengths = length.astype(np.int64)

    # LPT assignment: exactly BLOC sequences per core, balanced token sums
    order = np.argsort(-lengths, kind="stable")
    loads = np.zeros(NCORES, dtype=np.int64)
    counts = np.zeros(NCORES, dtype=np.int64)
    slots = [[] for _ in range(NCORES)]
    for idx in order:
        avail = [c for c in range(NCORES) if counts[c] < BLOC]
        c = min(avail, key=lambda cc: (int(loads[cc]), cc))
        slots[c].append(int(idx))
        loads[c] += int(lengths[idx])
        counts[c] += 1
    t_real = int(loads.max())
    n_tiles = max(1, -(-t_real // 512))
    t_common = n_tiles * 512
    tot_chunks = n_tiles * 4

    nc = _build_module(n_tiles)

    # host-side weight prep (shared across cores)
    w1h = W1.T.astype(np.float16)                                # [128, 512]
    w2h = np.ascontiguousarray(
        W2.T.reshape(4, 128, H2).transpose(1, 0, 2)
    ).astype(np.float16)                                         # [128, 4, 512]
    w3h = np.ascontiguousarray(
        W3.T.reshape(4, 128, E).transpose(1, 0, 2)
    ).astype(np.float16)                                         # [128, 4, 256]
    # permute LSTM gate rows [i, f, g, o] -> [i, f, o, g] so the three
    # sigmoid gates are contiguous in the gates vector
    gperm = np.concatenate(
        [np.arange(0, 2 * H), np.arange(3 * H, 4 * H), np.arange(2 * H, 3 * H)]
    )
    # pre-halve the sigmoid gates (first 3H rows after permute) so the
    # device applies one tanh(x) to every gate chunk
    gscale = np.concatenate(
        [np.full(3 * H, 0.5, np.float32), np.ones(H, np.float32)]
    )[:, None]
    W_ih_p = W_ih[gperm] * gscale
    W_hh_p = W_hh[gperm] * gscale
    wihh = np.ascontiguousarray(
        W_ih_p.T.reshape(2, 128, 4 * H).transpose(1, 0, 2)
    ).astype(np.float16)                                         # [128, 2, 1024]
    whhh = np.ascontiguousarray(
        W_hh_p.T.reshape(2, 128, 4 * H).transpose(1, 0, 2)
    ).astype(np.float16)
    b1h = np.ascontiguousarray(b1.reshape(4, 128).T).astype(np.float32)
    b2h = np.ascontiguousarray(b2.reshape(4, 128).T).astype(np.float32)
    bgv = (b_ih + b_hh + W_ih @ b3).astype(np.float32)[gperm] * gscale[:, 0]
    bg16h = bgv.reshape(1, 8 * 128).astype(np.float16)
    identh = np.eye(128, dtype=np.float32)
    ident16h = np.eye(128, dtype=np.float16)
    ones1h = np.ones((1, 128), dtype=np.float32)

    in_maps = []
    for c in range(NCORES):
        bidx = slots[c]  # batch index per slot
        offs = np.concatenate(([0], np.cumsum(lengths[bidx])))
        xT = np.zeros((128, t_common), dtype=np.float16)
        maskh = np.full((128, tot_chunks, BLOC), NEG, dtype=np.float32)
        w0 = np.zeros((128, tot_chunks, BLOC), dtype=np.float32)
        for j in range(BLOC):
            ln = int(lengths[bidx[j]])
            o0 = int(offs[j])
            xT[:, o0:o0 + ln] = state[bidx[j], :ln, :].T
            pos = np.arange(o0, o0 + ln)
            maskh[pos % 128, pos // 128, j] = -C1
            w0[pos % 128, pos // 128, j] = 1.0 / ln
        m = {
            "w1": w1h, "w2": w2h, "w3": w3h,
            "wih": wihh, "whh": whhh, "b1": b1h, "b2": b2h, "bg16": bg16h,
            "mask": maskh, "w0T": w0.astype(np.float16),
            "ident": identh, "ident16": ident16h, "ones1": ones1h,
        }
        for gi, g in enumerate(_xt_groups(n_tiles)):
            m[f"xtg{gi}"] = np.ascontiguousarray(
                xT[:, g[0] * 512:(g[-1] + 1) * 512]
            )
        in_maps.append(m)

    res = run_bass_kernel_spmd(nc, in_maps, list(range(NCORES)))

    out = np.zeros((B, E + H), dtype=np.float32)
    for c in range(NCORES):
        att = res.results[c]["att"] + b3[None, :].astype(np.float32)
        qt = res.results[c]["qt"]
        for j in range(BLOC):
            out[slots[c][j], :E] = att[j]
            out[slots[c][j], E:] = qt[j]
    return out
